# revision 12
# baseline (speedup 1.0000x reference)
"""Trainium2 Bass kernel for nn_AttributeAttn (dense_transformer, memory-bound).

Math (collapsed reference):
    u = W.T @ v; uh, ue = u[:H], u[H:]
    hv[n,b] = hidden[n,b,:] @ uh          # the big reduction
    ev[c,b] = enc[c,b,:] @ ue
    bias    = b @ v
    out[b,n,c] = softmax_c(tanh(hv[n,b] + ev[c,b] + bias))

Distribution: data-parallel over B (4 batches per core, 8 cores).

The problem is pure HBM streaming: per core the inputs are hidden 8MB,
W 3MB, enc 0.25MB (all bf16; the 2e-2 rel-err gate leaves ~100x slack
vs f32 and bf16 keeps the score error ~1e-3), output 0.5MB bf16 (host
upcasts). Every load is a fully-contiguous DMA (>=0.75MB reaches
~400GB/s of the per-core HBM bandwidth; small or strided descriptors
choke at <70%).

Device schedule per core (two HWDGE rings stream concurrently; the 16
SDMA engines round-robin between the ring rows at packet granularity):
  sync  : [vb+Wq0, Wq1, hid0, hid2, hid4, hid6a, hid6b]
  scalar: [Wq2, Wq3+enc, hid1, hid3, hid5, hid7a, hid7b]
W arrives in quarters so the u matmuls overlap the W stream; ucols is
ready before hid0 lands.  The last two hidden blocks stream in 0.5MB
halves to shorten the arrival tail.  Two f32 warm-up matmuls at the
head hold the PE's HAM clock gate at 2.4 GHz (matmul cost is
moving-free-size cycles; hidden = 32K columns = 13.7us of PE that must
run warm to hide under the ~29us stream), and a dummy Tanh preloads
the ACT table set during the stream.  Per n-block: contract over H in
PSUM, build the score tile in PSUM with TensorE (ones (x) evb + rank-1
hv broadcast), tanh, exp, row-sum, reciprocal, scale into a resident
output tile; the output leaves in two 0.25MB stores (gpsimd mid-stream,
sync at the tail).

Host side only shards/transposes/casts (no module math on host).
"""
import sys
import types

import numpy as np
import ml_dtypes

BF = ml_dtypes.bfloat16

# The container's antenv stub lacks axon_hooks; provide it so trace=True
# works when the test harness requests profiling. Harmless otherwise.
if "antenv.axon_hooks" not in sys.modules:
    _hooks_mod = types.ModuleType("antenv.axon_hooks")
    try:
        from trn_agent_boot.trn_boot import _ntff_profile_via_ctypes
        _ntff_hook = _ntff_profile_via_ctypes("/opt/axon/libaxon_pjrt.so")
    except Exception:
        _ntff_hook = None
    _hooks_mod.get_axon_ntff_profile_hook = lambda: _ntff_hook
    _hooks_mod.set_axon_ntff_profile_hook = lambda h: None
    sys.modules["antenv.axon_hooks"] = _hooks_mod

import concourse.bacc as bacc
import concourse.tile as tile
from concourse import mybir
from concourse.bass_utils import run_bass_kernel_spmd

f32 = mybir.dt.float32
bf16 = mybir.dt.bfloat16
AF = mybir.ActivationFunctionType
X = mybir.AxisListType.X
ADD = mybir.AluOpType.add
MUL = mybir.AluOpType.mult

N, B, H = 1024, 32, 1024
C, K = 64, 512
NCORES = 8
BPC = B // NCORES            # 4 batches per core
HC = H // 128                # 8 h-chunks
KC = K // 128                # 4 k-chunks
JC = (H + K) // 128          # 12 u columns
NBLK = N // 128              # 8 n-blocks per core
FW = BPC * C                 # 256 free (bb, c) elements per n-block
BW = 128 * BPC               # 512 hv free elements per n-block
WROW = H + K                 # 1536
WQ = 2 * WROW                # one W quarter (2 h-chunks) per partition
ENC_W = KC * FW              # 1024 enc columns per partition
HB = NBLK * HC * BW          # hid columns per partition

# Set by test harness to capture an NTFF profile.
TRACE = False
TRACE_KW = {}
LAST_RESULT = None

_cached = None


WV_W = 2 * HC + HC * WROW + ENC_W   # vb | w chunks 0..7 | enc


def _build():
    nc = bacc.Bacc(None, target_bir_lowering=False)
    wv_d = nc.dram_tensor("wv", [128, WV_W], bf16, kind="ExternalInput")
    hid_d = nc.dram_tensor("hid", [128, HB], bf16, kind="ExternalInput")
    out_d = nc.dram_tensor("out", [128, NBLK * FW], bf16, kind="ExternalOutput")

    with tile.TileContext(nc) as tc:
        with (
            tc.tile_pool(name="consts", bufs=1) as consts,
            tc.tile_pool(name="work", bufs=3) as work,
            tc.tile_pool(name="ps_warm", bufs=1, space="PSUM") as pw,
        ):
            # --- loads: W streams as 8 eighths alternating rings so the u
            # matmuls can chase arrivals at fine granularity ---
            rings = [nc.sync, nc.scalar]
            wv_sb = consts.tile([128, WV_W], bf16, tag="wv")
            cuts = [0] + [2 * HC + ic * WROW for ic in range(1, HC)] + [WV_W]
            for ic in range(HC):
                rings[ic % 2].dma_start(
                    out=wv_sb[:, cuts[ic]:cuts[ic + 1]],
                    in_=wv_d[:, cuts[ic]:cuts[ic + 1]])
            vb_sb = wv_sb[:, 0:2 * HC]
            enc_sb = wv_sb[:, 2 * HC + HC * WROW:]

            def wchunk(ic, j0, j1):
                off = 2 * HC + ic * WROW
                return wv_sb[:, off + j0:off + j1]

            hid_sb = []
            for k in range(NBLK - 2):
                t = consts.tile([128, HC * BW], bf16, tag=f"hid{k}")
                rings[k % 2].dma_start(
                    out=t, in_=hid_d[:, k * HC * BW:(k + 1) * HC * BW])
                hid_sb.append(t)
            halves = {}
            for hf in range(2):            # arrival order 6a, 7a, 6b, 7b
                for k in (6, 7):
                    t = consts.tile([128, HC * BW // 2], bf16,
                                    tag=f"hid{k}{hf}")
                    off = k * HC * BW + hf * (HC * BW // 2)
                    rings[k % 2].dma_start(
                        out=t, in_=hid_d[:, off:off + HC * BW // 2])
                    halves[k, hf] = t

            # --- PE warm-up + ACT table preload (both overlap the stream).
            # Two f32 matmuls (4 cyc/row, ~3.4us at the cold 1.2GHz clock)
            # hold the HAM activity window busy until real work arrives.
            warm_src = consts.tile([128, 512], f32, tag="warm_src")
            nc.vector.memset(warm_src, 1.0)
            ones_f = consts.tile([1, 128], f32, tag="ones_f")
            nc.vector.memset(ones_f, 1.0)
            ones = consts.tile([1, 128], bf16, tag="ones")
            nc.vector.tensor_copy(ones, ones_f)
            warm_bf = consts.tile([128, 512], bf16, tag="warm_bf")
            nc.vector.tensor_copy(warm_bf, warm_src)
            warm_ps = pw.tile([1, 512], f32, tag="warm")
            tpre = work.tile([1, 1], f32, tag="tpre")
            nc.scalar.activation(out=tpre, in_=warm_src[0:1, 0:1],
                                 func=AF.Tanh)

            def warm(n, wide=False):
                for _ in range(n):
                    if wide:
                        nc.tensor.matmul(warm_ps, warm_src[:, 0:1], warm_src,
                                         start=True, stop=True)
                    else:
                        nc.tensor.matmul(warm_ps, warm_bf[:, 0:1], warm_bf,
                                         start=True, stop=True)

            warm(1, wide=True)

            with tc.tile_pool(name="ps_setup", bufs=1, space="PSUM") as pset:
                # u row = v.T @ W (1, 1536), one W eighth at a time as the
                # stream delivers it; bf16 warm fillers plug the gaps.
                u_ps = pset.tile([1, 3, 512], f32, tag="u")
                bias_ps = pset.tile([1, 1], f32, tag="bias")
                for ic in range(HC):
                    for jb in range(3):
                        nc.tensor.matmul(
                            u_ps[:, jb, :],
                            vb_sb[:, ic:ic + 1],
                            wchunk(ic, jb * 512, (jb + 1) * 512),
                            start=(ic == 0), stop=(ic == HC - 1))
                    if ic == 0:
                        # bias = b @ v (8 rank-1s, ~1 cycle each; only
                        # needs vb which arrives with the first eighth)
                        for icb in range(HC):
                            nc.tensor.matmul(
                                bias_ps, vb_sb[:, icb:icb + 1],
                                vb_sb[:, HC + icb:HC + icb + 1],
                                start=(icb == 0), stop=(icb == HC - 1))
                    warm(1)
                bias_sb = consts.tile([1, 1], f32, tag="bias_sb")
                nc.vector.tensor_copy(bias_sb, bias_ps)

                u_row = consts.tile([1, JC, 128], bf16, tag="urow")
                nc.vector.tensor_copy(
                    u_row.rearrange("p a b -> p (a b)").rearrange(
                        "p (x y) -> p x y", x=3), u_ps)

                # u columns (128, 12) via rank-1 transposes
                uc_ps = pset.tile([128, JC], f32, tag="uc")
                for jc in range(JC):
                    nc.tensor.matmul(
                        uc_ps[:, jc:jc + 1], u_row[0:1, jc, :],
                        ones[:, 0:1], start=True, stop=True)
                ucols = consts.tile([128, JC], bf16, tag="ucols")
                nc.vector.tensor_copy(ucols, uc_ps)

                # ev row (1, 256) then evb = ev + bias, broadcast to all
                # partitions once (the per-block PE broadcast is hoisted)
                ev_ps = pset.tile([1, FW], f32, tag="ev")
                for kc in range(KC):
                    nc.tensor.matmul(
                        ev_ps, ucols[:, HC + kc:HC + kc + 1],
                        enc_sb[:, kc * FW:(kc + 1) * FW],
                        start=(kc == 0), stop=(kc == KC - 1))
                evb_row = consts.tile([1, FW], bf16, tag="evb")
                nc.vector.tensor_scalar_add(evb_row, ev_ps, bias_sb[:, 0:1])
                bc_ps = pset.tile([128, FW], f32, tag="bc")
                nc.tensor.matmul(bc_ps, ones, evb_row, start=True, stop=True)
                evb_rep = consts.tile([128, FW], f32, tag="evbrep")
                nc.vector.tensor_copy(evb_rep, bc_ps)
                warm(4)

            # --- per n-block: contract over H, rank-1 broadcast the hv row
            # into PSUM, add evb + tanh + exp + normalize into the resident
            # output tile.  The score/softmax of block k is deferred until
            # after block k+1's contraction so the PE never stalls on the
            # DVE row copy mid-stream.
            o_all = consts.tile([128, NBLK * FW], bf16, tag="o_all")
            with tc.tile_pool(name="ps_main", bufs=1, space="PSUM") as pp:
                rows = {}

                def hv_row(k, acc):
                    row = work.tile([1, BW], bf16, tag="row", bufs=3,
                                    name=f"row_{k}")
                    nc.vector.tensor_copy(row, acc)
                    rows[k] = row

                def score(k):
                    rowv = rows[k].rearrange("p (n bb) -> p bb n", bb=BPC)
                    sc_ps = pp.tile([128, FW], f32, tag="score", bufs=3,
                                    name=f"score_{k}")
                    for bb in range(BPC):
                        nc.tensor.matmul(
                            sc_ps[:, bb * C:(bb + 1) * C],
                            rowv[0:1, bb, :], ones[:, 0:C],
                            start=True, stop=True, skip_group_check=True)
                    sc = work.tile([128, FW], f32, tag="sc", bufs=2,
                                   name=f"sc_{k}")
                    nc.vector.scalar_tensor_tensor(
                        out=sc, in0=sc_ps, scalar=1.0, in1=evb_rep,
                        op0=MUL, op1=ADD)
                    nc.scalar.activation(out=sc, in_=sc, func=AF.Tanh)
                    nc.scalar.activation(out=sc, in_=sc, func=AF.Exp)
                    den = work.tile([128, BPC], f32, tag="den", bufs=2,
                                    name=f"den_{k}")
                    nc.vector.tensor_reduce(
                        den, sc.rearrange("p (bb c) -> p bb c", c=C),
                        axis=X, op=ADD)
                    nc.vector.reciprocal(den, den)
                    for bb in range(BPC):
                        nc.vector.tensor_scalar_mul(
                            o_all[:, k * FW + bb * C:k * FW + (bb + 1) * C],
                            sc[:, bb * C:(bb + 1) * C],
                            den[:, bb:bb + 1])

                pending = None
                for k in range(NBLK - 2):
                    acc = pp.tile([1, BW], f32, tag="acc", bufs=3,
                                  name=f"acc_{k}")
                    for hc in range(HC):
                        nc.tensor.matmul(
                            acc, ucols[:, hc:hc + 1],
                            hid_sb[k][:, hc * BW:(hc + 1) * BW],
                            start=(hc == 0), stop=(hc == HC - 1))
                    hv_row(k, acc)
                    if pending is not None:
                        score(pending)
                        if pending == 3:
                            # first output half leaves mid-stream on the
                            # otherwise-idle SWDGE path
                            nc.gpsimd.dma_start(
                                out=out_d[:, 0:4 * FW],
                                in_=o_all[:, 0:4 * FW])
                    pending = k

                # last two blocks stream in halves; hv contractions chase
                # the half-arrivals, score builds slot between them
                hw2 = BW // 2
                accs = {k: pp.tile([1, BW], f32, tag="acc", bufs=3,
                                   name=f"acc_{k}") for k in (6, 7)}

                def hv_half(k, hf):
                    for hc in range(HC):
                        nc.tensor.matmul(
                            accs[k][:, hf * hw2:(hf + 1) * hw2],
                            ucols[:, hc:hc + 1],
                            halves[k, hf][:, hc * hw2:(hc + 1) * hw2],
                            start=(hc == 0), stop=(hc == HC - 1),
                            skip_group_check=True)

                hv_half(6, 0)
                hv_half(7, 0)
                score(pending)
                hv_half(6, 1)
                hv_row(6, accs[6])
                hv_half(7, 1)
                hv_row(7, accs[7])
                score(6)
                score(7)
                nc.sync.dma_start(
                    out=out_d[:, 4 * FW:], in_=o_all[:, 4 * FW:])
    nc.compile()
    return nc


def kernel(**inputs):
    global _cached, LAST_RESULT
    hidden = np.asarray(inputs["hidden"], dtype=np.float32)
    enc = np.asarray(inputs["encoder_outputs"], dtype=np.float32)
    W = np.asarray(inputs["W"], dtype=np.float32)
    b = np.asarray(inputs["b"], dtype=np.float32)
    v = np.asarray(inputs["v"], dtype=np.float32)

    if _cached is None:
        _cached = _build()
    nc = _cached

    # vb: column ic holds v[ic*128:(ic+1)*128]; column HC+ic holds b chunk.
    vb = np.concatenate(
        [v.reshape(HC, 128).T, b.reshape(HC, 128).T], axis=1).astype(BF)
    # W partition-major chunks: wt[p, ic, j] = W[ic*128 + p, j]
    wt = W.astype(BF).reshape(HC, 128, WROW).transpose(1, 0, 2)
    wv_head = np.concatenate([vb, wt.reshape(128, HC * WROW)], axis=1)

    hb = hidden.astype(BF)
    eb = enc.astype(BF)

    in_maps = []
    for j in range(NCORES):
        bsl = slice(j * BPC, (j + 1) * BPC)
        # hid: [p, (k, hc, fn*BPC+bb)]; blocks 6/7 split into halves
        # [p, (k, half, hc, f)] so the stream tail arrives in 0.5MB steps.
        x = hb[:, bsl, :]                                   # (N, BPC, H)
        x = x.transpose(2, 0, 1)                            # (H, N, BPC)
        x = x.reshape(HC, 128, NBLK, BW)                    # (hc, p, k, f)
        full = x[:, :, :NBLK - 2].transpose(1, 2, 0, 3)     # (p, k, hc, f)
        tail = x[:, :, NBLK - 2:].reshape(HC, 128, 2, 2, BW // 2)
        tail = tail.transpose(1, 2, 3, 0, 4)                # (p,k,half,hc,f)
        hid_t = np.ascontiguousarray(np.concatenate(
            [full.reshape(128, -1), tail.reshape(128, -1)], axis=1))
        # enc: [p, kc*FW + bb*C + c]
        e = eb[:, bsl, :].transpose(2, 1, 0)                # (K, BPC, C)
        e = e.reshape(KC, 128, FW).transpose(1, 0, 2)
        enc_t = e.reshape(128, ENC_W)
        wv = np.ascontiguousarray(np.concatenate([wv_head, enc_t], axis=1))
        in_maps.append({"hid": hid_t, "wv": wv})

    res = run_bass_kernel_spmd(
        nc, in_maps, core_ids=list(range(NCORES)), trace=TRACE, **TRACE_KW)
    LAST_RESULT = res

    out = np.empty((B, N, C), dtype=np.float32)
    for j in range(NCORES):
        o = res.results[j]["out"].astype(np.float32)
        o = o.reshape(128, NBLK, BPC, C).transpose(2, 1, 0, 3)
        out[j * BPC:(j + 1) * BPC] = o.reshape(BPC, N, C)
    return out


# revision 17
# speedup vs baseline: 1.1019x; 1.1019x over previous
"""Trainium2 Bass kernel for nn_AttributeAttn (dense_transformer, memory-bound).

Math (collapsed reference):
    u = W.T @ v; uh, ue = u[:H], u[H:]
    hv[n,b] = hidden[n,b,:] @ uh          # the big reduction
    ev[c,b] = enc[c,b,:] @ ue
    bias    = b @ v
    out[b,n,c] = softmax_c(tanh(hv[n,b] + ev[c,b] + bias))

Distribution: data-parallel over B (4 batches per core, 8 cores).

The problem is pure HBM streaming: per core the inputs are hidden 8MB,
W 3MB, enc 0.25MB (all bf16; the 2e-2 rel-err gate leaves ~100x slack
vs f32 and bf16 keeps the score error ~1e-3), output 0.5MB bf16 (host
upcasts). Every load is a fully-contiguous DMA (>=0.75MB reaches
~400GB/s of the per-core HBM bandwidth; small or strided descriptors
choke at <70%).

Device schedule per core (two HWDGE rings stream concurrently; the 16
SDMA engines round-robin between the ring rows at packet granularity):
  sync  : [vb+Wq0, Wq1, hid0, hid2, hid4, hid6a, hid6b]
  scalar: [Wq2, Wq3+enc, hid1, hid3, hid5, hid7a, hid7b]
W arrives in quarters so the u matmuls overlap the W stream; ucols is
ready before hid0 lands.  The last two hidden blocks stream in 0.5MB
halves to shorten the arrival tail.  Two f32 warm-up matmuls at the
head hold the PE's HAM clock gate at 2.4 GHz (matmul cost is
moving-free-size cycles; hidden = 32K columns = 13.7us of PE that must
run warm to hide under the ~29us stream), and a dummy Tanh preloads
the ACT table set during the stream.  Per n-block: contract over H in
PSUM, build the score tile in PSUM with TensorE (ones (x) evb + rank-1
hv broadcast), tanh, exp, row-sum, reciprocal, scale into a resident
output tile; the output leaves in two 0.25MB stores (gpsimd mid-stream,
sync at the tail).

Host side only shards/transposes/casts (no module math on host).
"""
import sys
import types

import numpy as np
import ml_dtypes

BF = ml_dtypes.bfloat16

# The container's antenv stub lacks axon_hooks; provide it so trace=True
# works when the test harness requests profiling. Harmless otherwise.
if "antenv.axon_hooks" not in sys.modules:
    _hooks_mod = types.ModuleType("antenv.axon_hooks")
    try:
        from trn_agent_boot.trn_boot import _ntff_profile_via_ctypes
        _ntff_hook = _ntff_profile_via_ctypes("/opt/axon/libaxon_pjrt.so")
    except Exception:
        _ntff_hook = None
    _hooks_mod.get_axon_ntff_profile_hook = lambda: _ntff_hook
    _hooks_mod.set_axon_ntff_profile_hook = lambda h: None
    sys.modules["antenv.axon_hooks"] = _hooks_mod

import concourse.bacc as bacc
import concourse.tile as tile
from concourse import mybir
from concourse.bass_utils import run_bass_kernel_spmd

f32 = mybir.dt.float32
bf16 = mybir.dt.bfloat16
AF = mybir.ActivationFunctionType
X = mybir.AxisListType.X
ADD = mybir.AluOpType.add
MUL = mybir.AluOpType.mult

N, B, H = 1024, 32, 1024
C, K = 64, 512
NCORES = 8
BPC = B // NCORES            # 4 batches per core
HC = H // 128                # 8 h-chunks
KC = K // 128                # 4 k-chunks
JC = (H + K) // 128          # 12 u columns
NBLK = N // 128              # 8 n-blocks per core
FW = BPC * C                 # 256 free (bb, c) elements per n-block
BW = 128 * BPC               # 512 hv free elements per n-block
WROW = H + K                 # 1536
WQ = 2 * WROW                # one W quarter (2 h-chunks) per partition
ENC_W = KC * FW              # 1024 enc columns per partition
HB = NBLK * HC * BW          # hid columns per partition

# Set by test harness to capture an NTFF profile.
TRACE = False
TRACE_KW = {}
LAST_RESULT = None

_cached = None


WV_W = 2 * HC + HC * WROW + ENC_W   # vb | w chunks 0..7 | enc


def _build():
    nc = bacc.Bacc(None, target_bir_lowering=False)
    wv_d = nc.dram_tensor("wv", [128, WV_W], bf16, kind="ExternalInput")
    hid_d = nc.dram_tensor("hid", [128, HB], bf16, kind="ExternalInput")
    out_d = nc.dram_tensor("out", [128, NBLK * FW], bf16, kind="ExternalOutput")

    with tile.TileContext(nc) as tc:
        with (
            tc.tile_pool(name="consts", bufs=1) as consts,
            tc.tile_pool(name="work", bufs=3) as work,
            tc.tile_pool(name="ps_warm", bufs=1, space="PSUM") as pw,
        ):
            # --- loads: W in two ~1.7MB halves, one per ring (12KB/partition
            # descriptors stream at full ring rate; finer splits pay a
            # ~0.6-1us ring bubble per transfer and land LATER) ---
            rings = [nc.sync, nc.scalar]
            wv_sb = consts.tile([128, WV_W], bf16, tag="wv")
            mid = 2 * HC + (HC // 2) * WROW
            nc.sync.dma_start(out=wv_sb[:, :mid], in_=wv_d[:, :mid])
            nc.scalar.dma_start(out=wv_sb[:, mid:], in_=wv_d[:, mid:])
            vb_sb = wv_sb[:, 0:2 * HC]
            enc_sb = wv_sb[:, 2 * HC + HC * WROW:]

            def wchunk(ic, j0, j1):
                off = 2 * HC + ic * WROW
                return wv_sb[:, off + j0:off + j1]

            hid_sb = []
            for k in range(NBLK - 2):
                t = consts.tile([128, HC * BW], bf16, tag=f"hid{k}")
                rings[k % 2].dma_start(
                    out=t, in_=hid_d[:, k * HC * BW:(k + 1) * HC * BW])
                hid_sb.append(t)
            halves = {}
            for hf in range(2):            # arrival order 6a, 7a, 6b, 7b
                for k in (6, 7):
                    t = consts.tile([128, HC * BW // 2], bf16,
                                    tag=f"hid{k}{hf}")
                    off = k * HC * BW + hf * (HC * BW // 2)
                    rings[k % 2].dma_start(
                        out=t, in_=hid_d[:, off:off + HC * BW // 2])
                    halves[k, hf] = t

            # --- PE warm-up + ACT table preload (both overlap the stream).
            # Two f32 matmuls (4 cyc/row, ~3.4us at the cold 1.2GHz clock)
            # hold the HAM activity window busy until real work arrives.
            warm_src = consts.tile([128, 512], f32, tag="warm_src")
            nc.vector.memset(warm_src, 1.0)
            ones_f = consts.tile([1, 128], f32, tag="ones_f")
            nc.vector.memset(ones_f, 1.0)
            ones = consts.tile([1, 128], bf16, tag="ones")
            nc.vector.tensor_copy(ones, ones_f)
            warm_bf = consts.tile([128, 512], bf16, tag="warm_bf")
            nc.vector.tensor_copy(warm_bf, warm_src)
            warm_ps = pw.tile([1, 512], f32, tag="warm")
            tpre = work.tile([1, 1], f32, tag="tpre")
            nc.scalar.activation(out=tpre, in_=warm_src[0:1, 0:1],
                                 func=AF.Tanh)

            def warm(n, wide=False):
                for _ in range(n):
                    if wide:
                        nc.tensor.matmul(warm_ps, warm_src[:, 0:1], warm_src,
                                         start=True, stop=True)
                    else:
                        nc.tensor.matmul(warm_ps, warm_bf[:, 0:1], warm_bf,
                                         start=True, stop=True)

            # ~4.6us of f32 warm-up keeps the HAM activity window busy from
            # program start until the W halves land (~15us), so the u phase
            # runs at the full 2.4GHz clock.
            warm(5, wide=True)

            with tc.tile_pool(name="ps_setup", bufs=1, space="PSUM") as pset:
                # u row = v.T @ W (1, 1536)
                u_ps = pset.tile([1, 3, 512], f32, tag="u")
                bias_ps = pset.tile([1, 1], f32, tag="bias")
                for ic in range(HC):
                    for jb in range(3):
                        nc.tensor.matmul(
                            u_ps[:, jb, :],
                            vb_sb[:, ic:ic + 1],
                            wchunk(ic, jb * 512, (jb + 1) * 512),
                            start=(ic == 0), stop=(ic == HC - 1))
                    if ic == 0:
                        # bias = b @ v (8 rank-1s, ~1 cycle each)
                        for icb in range(HC):
                            nc.tensor.matmul(
                                bias_ps, vb_sb[:, icb:icb + 1],
                                vb_sb[:, HC + icb:HC + icb + 1],
                                start=(icb == 0), stop=(icb == HC - 1))
                bias_sb = consts.tile([1, 1], f32, tag="bias_sb")
                nc.vector.tensor_copy(bias_sb, bias_ps)

                u_row = consts.tile([1, JC, 128], bf16, tag="urow")
                nc.vector.tensor_copy(
                    u_row.rearrange("p a b -> p (a b)").rearrange(
                        "p (x y) -> p x y", x=3), u_ps)

                # u columns (128, 12) via rank-1 transposes
                uc_ps = pset.tile([128, JC], f32, tag="uc")
                for jc in range(JC):
                    nc.tensor.matmul(
                        uc_ps[:, jc:jc + 1], u_row[0:1, jc, :],
                        ones[:, 0:1], start=True, stop=True)
                ucols = consts.tile([128, JC], bf16, tag="ucols")
                nc.vector.tensor_copy(ucols, uc_ps)

                # ev row (1, 256) then evb = ev + bias, broadcast to all
                # partitions once (the per-block PE broadcast is hoisted)
                ev_ps = pset.tile([1, FW], f32, tag="ev")
                for kc in range(KC):
                    nc.tensor.matmul(
                        ev_ps, ucols[:, HC + kc:HC + kc + 1],
                        enc_sb[:, kc * FW:(kc + 1) * FW],
                        start=(kc == 0), stop=(kc == KC - 1))
                evb_row = consts.tile([1, FW], bf16, tag="evb")
                nc.vector.tensor_scalar_add(evb_row, ev_ps, bias_sb[:, 0:1])
                bc_ps = pset.tile([128, FW], f32, tag="bc")
                nc.tensor.matmul(bc_ps, ones, evb_row, start=True, stop=True)
                evb_rep = consts.tile([128, FW], f32, tag="evbrep")
                nc.vector.tensor_copy(evb_rep, bc_ps)
                warm(4)

            # --- per n-block: contract over H, rank-1 broadcast the hv row
            # into PSUM, add evb + tanh + exp + normalize into the resident
            # output tile.  The score/softmax of block k is deferred until
            # after block k+1's contraction so the PE never stalls on the
            # DVE row copy mid-stream.
            o_all = consts.tile([128, NBLK * FW], bf16, tag="o_all")
            with tc.tile_pool(name="ps_main", bufs=1, space="PSUM") as pp:
                rows = {}

                def hv_row(k, acc):
                    row = work.tile([1, BW], bf16, tag="row", bufs=3,
                                    name=f"row_{k}")
                    nc.vector.tensor_copy(row, acc)
                    rows[k] = row

                def score(k):
                    rowv = rows[k].rearrange("p (n bb) -> p bb n", bb=BPC)
                    sc_ps = pp.tile([128, FW], f32, tag="score", bufs=3,
                                    name=f"score_{k}")
                    for bb in range(BPC):
                        nc.tensor.matmul(
                            sc_ps[:, bb * C:(bb + 1) * C],
                            rowv[0:1, bb, :], ones[:, 0:C],
                            start=True, stop=True, skip_group_check=True)
                    sc = work.tile([128, FW], f32, tag="sc", bufs=2,
                                   name=f"sc_{k}")
                    nc.vector.scalar_tensor_tensor(
                        out=sc, in0=sc_ps, scalar=1.0, in1=evb_rep,
                        op0=MUL, op1=ADD)
                    nc.scalar.activation(out=sc, in_=sc, func=AF.Tanh)
                    nc.scalar.activation(out=sc, in_=sc, func=AF.Exp)
                    den = work.tile([128, BPC], f32, tag="den", bufs=2,
                                    name=f"den_{k}")
                    nc.vector.tensor_reduce(
                        den, sc.rearrange("p (bb c) -> p bb c", c=C),
                        axis=X, op=ADD)
                    nc.vector.reciprocal(den, den)
                    for bb in range(BPC):
                        nc.vector.tensor_scalar_mul(
                            o_all[:, k * FW + bb * C:k * FW + (bb + 1) * C],
                            sc[:, bb * C:(bb + 1) * C],
                            den[:, bb:bb + 1])

                pending = None
                for k in range(NBLK - 2):
                    acc = pp.tile([1, BW], f32, tag="acc", bufs=3,
                                  name=f"acc_{k}")
                    for hc in range(HC):
                        nc.tensor.matmul(
                            acc, ucols[:, hc:hc + 1],
                            hid_sb[k][:, hc * BW:(hc + 1) * BW],
                            start=(hc == 0), stop=(hc == HC - 1))
                    hv_row(k, acc)
                    if pending is not None:
                        score(pending)
                        if pending == 3:
                            # first output half leaves mid-stream on the
                            # otherwise-idle SWDGE path
                            nc.gpsimd.dma_start(
                                out=out_d[:, 0:4 * FW],
                                in_=o_all[:, 0:4 * FW])
                    pending = k

                # last two blocks stream in halves; hv contractions chase
                # the half-arrivals, score builds slot between them
                hw2 = BW // 2
                accs = {k: pp.tile([1, BW], f32, tag="acc", bufs=3,
                                   name=f"acc_{k}") for k in (6, 7)}

                def hv_half(k, hf):
                    for hc in range(HC):
                        nc.tensor.matmul(
                            accs[k][:, hf * hw2:(hf + 1) * hw2],
                            ucols[:, hc:hc + 1],
                            halves[k, hf][:, hc * hw2:(hc + 1) * hw2],
                            start=(hc == 0), stop=(hc == HC - 1),
                            skip_group_check=True)

                hv_half(6, 0)
                hv_half(7, 0)
                score(pending)
                hv_half(6, 1)
                hv_row(6, accs[6])
                score(6)
                hv_half(7, 1)
                hv_row(7, accs[7])
                score(7)
                nc.sync.dma_start(
                    out=out_d[:, 4 * FW:], in_=o_all[:, 4 * FW:])
    nc.compile()
    return nc


def kernel(**inputs):
    global _cached, LAST_RESULT
    hidden = np.asarray(inputs["hidden"], dtype=np.float32)
    enc = np.asarray(inputs["encoder_outputs"], dtype=np.float32)
    W = np.asarray(inputs["W"], dtype=np.float32)
    b = np.asarray(inputs["b"], dtype=np.float32)
    v = np.asarray(inputs["v"], dtype=np.float32)

    if _cached is None:
        _cached = _build()
    nc = _cached

    # vb: column ic holds v[ic*128:(ic+1)*128]; column HC+ic holds b chunk.
    vb = np.concatenate(
        [v.reshape(HC, 128).T, b.reshape(HC, 128).T], axis=1).astype(BF)
    # W partition-major chunks: wt[p, ic, j] = W[ic*128 + p, j]
    wt = W.astype(BF).reshape(HC, 128, WROW).transpose(1, 0, 2)
    wv_head = np.concatenate([vb, wt.reshape(128, HC * WROW)], axis=1)

    hb = hidden.astype(BF)
    eb = enc.astype(BF)

    in_maps = []
    for j in range(NCORES):
        bsl = slice(j * BPC, (j + 1) * BPC)
        # hid: [p, (k, hc, fn*BPC+bb)]; blocks 6/7 split into halves
        # [p, (k, half, hc, f)] so the stream tail arrives in 0.5MB steps.
        x = hb[:, bsl, :]                                   # (N, BPC, H)
        x = x.transpose(2, 0, 1)                            # (H, N, BPC)
        x = x.reshape(HC, 128, NBLK, BW)                    # (hc, p, k, f)
        full = x[:, :, :NBLK - 2].transpose(1, 2, 0, 3)     # (p, k, hc, f)
        tail = x[:, :, NBLK - 2:].reshape(HC, 128, 2, 2, BW // 2)
        tail = tail.transpose(1, 2, 3, 0, 4)                # (p,k,half,hc,f)
        hid_t = np.ascontiguousarray(np.concatenate(
            [full.reshape(128, -1), tail.reshape(128, -1)], axis=1))
        # enc: [p, kc*FW + bb*C + c]
        e = eb[:, bsl, :].transpose(2, 1, 0)                # (K, BPC, C)
        e = e.reshape(KC, 128, FW).transpose(1, 0, 2)
        enc_t = e.reshape(128, ENC_W)
        wv = np.ascontiguousarray(np.concatenate([wv_head, enc_t], axis=1))
        in_maps.append({"hid": hid_t, "wv": wv})

    res = run_bass_kernel_spmd(
        nc, in_maps, core_ids=list(range(NCORES)), trace=TRACE, **TRACE_KW)
    LAST_RESULT = res

    out = np.empty((B, N, C), dtype=np.float32)
    for j in range(NCORES):
        o = res.results[j]["out"].astype(np.float32)
        o = o.reshape(128, NBLK, BPC, C).transpose(2, 1, 0, 3)
        out[j * BPC:(j + 1) * BPC] = o.reshape(BPC, N, C)
    return out
